# revision 5
# baseline (speedup 1.0000x reference)
"""Trainium2 Bass kernel for nn_AFMExpert (AFM attention-over-pairs net).

Math (per batch b):
    had[p, e]  = x[b, i_p, e] * x[b, j_p, e]          P = F*(F-1)/2 pairs
    a          = relu(had @ W1^T + b1)                 [P, NHID]
    logits     = a @ W2^T + b2                         [P, OUT]
    attn       = softmax(logits, axis=pairs)
    afm[e]     = sum_p attn[p, e] * had[p, e]          (OUT == E)
    out[b]     = afm @ pw^T + pb                       [1]

Distribution: pure data parallel, batch sharded 8 ways (64 batches/core),
weights replicated.  No collectives.

Per-core layout ("L1"): two batches stacked on the 128 SBUF partitions
(partition = t*64 + e, t in {0,1}), pair dim + batch-pair dim on the free
axis.  Pairs are enumerated by rotation diagonals d in 1..31 (64 pairs
each) plus d=32 (32 pairs); the host ships x pre-rotated (xrot) so `had`
generation is 3 big DVE tensor_mul ops per 4-batch-pair group, using
stride-tricked access patterns (stride-0 broadcast of the base block, a
stride-2 overlapping-window walk over the rotated block) that run in the
DVE's 2x bf16 perf mode.

Per batch-pair slot:
  mm1 (PE, bf16, block-diag [128,128] lhsT) -> ps1 f32 (2x 2-bank tiles)
  relu evac (1x PSUM read): ACT activation(Relu, bias) or DVE
     tensor_scalar(add bias, max 0) -- the ACT/DVE split is a balance knob
  mm2 -> ps2 f32 (2x 2-bank tiles)
  exp (ACT, 1x): activation(Exp, bias) PSUM -> SBUF *bf16*, softmax
     normalizer Z accumulated free via accum_out
  prod = exp * had: plain tensor_tensor, all-bf16 SBUF => DVE 2x mode
  S    = sum(prod): tensor_scalar(mult 1.0, accum_out), single-src
     all-SBUF bf16 => DVE 4x mode
Everything downstream needs only S and Z: out = sum_e pw*S/Z + pb.

Measured on 8 axon trn2 cores (baseline v1: ~187us).
"""

import os
import sys

for _p in ("/opt/trn_rl_repo", "/opt/pypackages"):
    if os.path.isdir(_p) and _p not in sys.path:
        sys.path.append(_p)

from contextlib import ExitStack

import ml_dtypes
import numpy as np

import concourse.bass as bass
import concourse.mybir as mybir
import concourse.tile as tile
from concourse import bacc
from concourse.bass_utils import run_bass_kernel_spmd

BF16 = mybir.dt.bfloat16
F32 = mybir.dt.float32

B, F, E, NHID, OUT = 512, 64, 64, 64, 64
NCORES = 8
BLOC = B // NCORES          # 64 batches per core
NPAIR = BLOC // 2           # 32 batch-pairs per core
PF = 2048                   # padded pair-block width
NREAL = 2016                # real pairs = F*(F-1)/2
XW = 192                    # xrot row width: 96 even-base + 96 odd-base cols
GN = 4                      # batch-pairs per had group
NG = NPAIR // GN            # 8 groups
PH = 1024                   # half width (2 PSUM banks of f32)

# pairs whose relu evacuation runs on DVE instead of ACT (engine balance)
RELU_DVE_PAIRS = frozenset((2, 5, 8, 11, 14, 17, 20, 23, 26, 29, 31))


def _win_ap(xg, gi_n, start, step, cnt):
    """Overlapping-window AP over the rotated-x tile: [128, gn, cnt, 64]
    where dim 2 walks `cnt` windows of 64 cols spaced `step` apart."""
    w = xg[:, :, start:start + 64].unsqueeze(2).broadcast_to(
        [128, gi_n, cnt, 64])
    lst = w.ap
    lst[2] = [step, cnt]
    w.ap = lst
    return w


def _build_nc():
    # Bacc (not raw Bass): its finalize() runs generate_event_semaphores,
    # which splits multi-wait sync_info onto InstEventSemaphore — the TRN2
    # ISA allows at most 1 sync wait per regular instruction.
    nc = bacc.Bacc(None)

    xrot = nc.declare_dram_parameter("xrot", [128, NPAIR * XW], BF16, isOutput=False)
    w1s_d = nc.declare_dram_parameter("w1s", [128, 128], BF16, isOutput=False)
    w2s_d = nc.declare_dram_parameter("w2s", [128, 128], BF16, isOutput=False)
    b1s_d = nc.declare_dram_parameter("b1s", [128, 1], F32, isOutput=False)
    b2s_d = nc.declare_dram_parameter("b2s", [128, 1], F32, isOutput=False)
    pws_d = nc.declare_dram_parameter("pws", [128, 1], F32, isOutput=False)
    mask_d = nc.declare_dram_parameter("mask", [128, 2], F32, isOutput=False)
    pb_d = nc.declare_dram_parameter("pb", [2, 1], F32, isOutput=False)
    out_d = nc.declare_dram_parameter("out", [BLOC, 1], F32, isOutput=True)

    with tile.TileContext(nc) as tc, ExitStack() as ctx:
        const = ctx.enter_context(tc.tile_pool(name="const", bufs=1))
        hadp = ctx.enter_context(tc.tile_pool(name="hadp", bufs=2))
        aring = ctx.enter_context(tc.tile_pool(name="aring", bufs=2))
        ering = ctx.enter_context(tc.tile_pool(name="ering", bufs=3))
        pring = ctx.enter_context(tc.tile_pool(name="pring", bufs=2))
        small = ctx.enter_context(tc.tile_pool(name="small", bufs=1))
        ps1p = ctx.enter_context(tc.tile_pool(name="ps1p", bufs=2, space="PSUM"))
        ps2p = ctx.enter_context(tc.tile_pool(name="ps2p", bufs=2, space="PSUM"))

        # ---- constants / inputs to SBUF ----
        # xrot is DMA'd per had-group so group 0's had-generation can
        # start early instead of waiting for the full transfer
        xr_tiles = []
        for g in range(NG):
            xr_g = const.tile([128, GN, XW], BF16, tag=f"xr{g}", name=f"xr{g}")
            nc.sync.dma_start(
                out=xr_g[:].rearrange("p a b -> p (a b)"),
                in_=xrot[:, g * GN * XW:(g + 1) * GN * XW])
            xr_tiles.append(xr_g)
        w1s = const.tile([128, 128], BF16, tag="w1s")
        nc.sync.dma_start(out=w1s[:], in_=w1s_d[:, :])
        w2s = const.tile([128, 128], BF16, tag="w2s")
        nc.sync.dma_start(out=w2s[:], in_=w2s_d[:, :])
        b1s = const.tile([128, 1], F32, tag="b1s")
        nc.sync.dma_start(out=b1s[:], in_=b1s_d[:, :])
        b2s = const.tile([128, 1], F32, tag="b2s")
        nc.sync.dma_start(out=b2s[:], in_=b2s_d[:, :])
        pws = const.tile([128, 1], F32, tag="pws")
        nc.sync.dma_start(out=pws[:], in_=pws_d[:, :])
        mask = const.tile([128, 2], F32, tag="mask")
        nc.sync.dma_start(out=mask[:], in_=mask_d[:, :])
        pb = const.tile([2, 1], F32, tag="pb")
        nc.sync.dma_start(out=pb[:], in_=pb_d[:, :])

        # accumulators: Z per (pair, psum-half), S per pair
        Zs2 = small.tile([128, 2 * NPAIR], F32, tag="Zs2")
        Ss = small.tile([128, NPAIR], F32, tag="Ss")

        # ---- had generation: 3 fused DVE ops per group ----
        def had_ops(had_t, g):
            """Thunks for one group's had generation (interleaved between
            the previous group's pairs so the DVE pipelines them)."""
            xg = xr_tiles[g]

            def evens():
                out = had_t[:].rearrange(
                    "p g (d x) -> p g d x", x=128)[:, :, 0:15, 64:128]
                in0 = xg[:, :, 0:64].unsqueeze(2).broadcast_to(
                    [128, GN, 15, 64])
                nc.vector.tensor_mul(out, in0, _win_ap(xg, GN, 2, 2, 15))

            def odds():
                out = had_t[:].rearrange(
                    "p g (d x) -> p g d x", x=128)[:, :, 0:16, 0:64]
                in0 = xg[:, :, 0:64].unsqueeze(2).broadcast_to(
                    [128, GN, 16, 64])
                nc.vector.tensor_mul(out, in0, _win_ap(xg, GN, 96, 2, 16))

            def d32():
                nc.vector.tensor_mul(
                    had_t[:, :, 1984:2016], xg[:, :, 0:32], xg[:, :, 32:64])

            return [evens, odds, d32]

        had_tiles = [hadp.tile([128, GN, PF], BF16, tag="had", name=f"had{g}")
                     for g in range(2)]

        def get_had(g):
            return had_tiles[g % 2]

        def do_pair(i, had_g, li):
            # mm1: block-diag lhsT, 4 chunks into two 2-bank PSUM tiles
            ps1a = ps1p.tile([128, PH], F32, tag="ps1h")
            ps1b = ps1p.tile([128, PH], F32, tag="ps1h")
            for c, pst in ((0, ps1a), (1, ps1a), (2, ps1b), (3, ps1b)):
                w = 480 if c == 3 else 512
                s = 512 * c
                nc.tensor.matmul(pst[:, s - 1024 * (c // 2):s - 1024 * (c // 2) + w],
                                 w1s[:], had_g[:, li, s:s + w],
                                 start=True, stop=True)

            a_sb = aring.tile([128, PF], BF16, tag="a")
            for h, pst in ((0, ps1a), (1, ps1b)):
                w = PH if h == 0 else NREAL - PH
                if i in RELU_DVE_PAIRS:
                    nc.vector.tensor_scalar(
                        out=a_sb[:, PH * h:PH * h + w], in0=pst[:, 0:w],
                        scalar1=b1s[:], scalar2=0.0,
                        op0=mybir.AluOpType.add, op1=mybir.AluOpType.max,
                    )
                else:
                    nc.scalar.activation(
                        a_sb[:, PH * h:PH * h + w], pst[:, 0:w],
                        mybir.ActivationFunctionType.Relu,
                        bias=b1s[:], scale=1.0,
                    )

            # mm2 + exp: exp reads PSUM f32, writes SBUF bf16; softmax
            # normalizer Z accumulated via accum_out
            exp_sb = ering.tile([128, PF], BF16, tag="e")
            for h in range(2):
                ps2 = ps2p.tile([128, PH], F32, tag="ps2h")
                for c in range(2):
                    s = PH * h + 512 * c
                    w = 480 if (h, c) == (1, 1) else 512
                    nc.tensor.matmul(ps2[:, 512 * c:512 * c + w], w2s[:],
                                     a_sb[:, s:s + w], start=True, stop=True)
                w = PH if h == 0 else NREAL - PH
                nc.scalar.activation(
                    exp_sb[:, PH * h:PH * h + w], ps2[:, 0:w],
                    mybir.ActivationFunctionType.Exp,
                    bias=b2s[:], scale=1.0,
                    accum_out=Zs2[:, 2 * i + h:2 * i + h + 1],
                )

            # attn-weighted sum: all-bf16 SBUF ops => DVE 2x / 4x modes
            prod = pring.tile([128, PF], BF16, tag="prod")
            nc.vector.tensor_mul(prod[:, 0:NREAL], exp_sb[:, 0:NREAL],
                                 had_g[:, li, 0:NREAL])
            nc.vector.tensor_scalar(
                out=prod[:, 0:NREAL], in0=prod[:, 0:NREAL],
                scalar1=1.0, scalar2=None, op0=mybir.AluOpType.mult,
                op1=mybir.AluOpType.add,
                accum_out=Ss[:, i:i + 1],
            )

        # group 0's had-generation runs up front; group g+1's 3 ops are
        # emitted after the prod/S of the first 3 pairs of group g
        for op in had_ops(had_tiles[0], 0):
            op()
        for g in range(NG):
            had_g = get_had(g)
            nxt = had_ops(get_had(g + 1), g + 1) if g + 1 < NG else []
            for li in range(GN):
                do_pair(g * GN + li, had_g, li)
                if li < len(nxt):
                    nxt[li]()

        # ---- finalize ----
        Zs2v = Zs2[:].rearrange("p (i h) -> p i h", h=2)
        Zs = small.tile([128, NPAIR], F32, tag="Zs")
        nc.vector.tensor_add(Zs[:], Zs2v[:, :, 0], Zs2v[:, :, 1])
        rz = small.tile([128, NPAIR], F32, tag="rz")
        nc.vector.reciprocal(rz[:], Zs[:])
        v = small.tile([128, NPAIR], F32, tag="v")
        nc.vector.tensor_mul(v[:], Ss[:], rz[:])
        v2 = small.tile([128, NPAIR], F32, tag="v2")
        nc.vector.tensor_scalar_mul(v2[:], v[:], pws[:])

        # partition-halves reduction via mask matmul: [2, NPAIR]
        fin_ps = ps1p.tile([2, NPAIR], F32, tag="ps1h")
        nc.tensor.matmul(fin_ps[:], mask[:], v2[:], start=True, stop=True)
        fin_sb = small.tile([2, NPAIR], F32, tag="fin")
        nc.vector.tensor_scalar(
            out=fin_sb[:], in0=fin_ps[:], scalar1=pb[:], scalar2=None,
            op0=mybir.AluOpType.add,
        )
        nc.sync.dma_start(
            out=out_d[:].rearrange("(i t) o -> t (i o)", t=2),
            in_=fin_sb[:],
        )

    nc.finalize()
    return nc


_NC = None


def _get_nc():
    global _NC
    if _NC is None:
        _NC = _build_nc()
    return _NC


def _prep_in_maps(inputs):
    bf = ml_dtypes.bfloat16
    x = np.asarray(inputs["x"], np.float32)          # [B, F, E]
    w1 = np.asarray(inputs["attn_w_w"], np.float32)  # [NHID, E]
    b1 = np.asarray(inputs["attn_w_b"], np.float32)  # [NHID]
    w2 = np.asarray(inputs["attn_h_w"], np.float32)  # [OUT, NHID]
    b2 = np.asarray(inputs["attn_h_b"], np.float32)  # [OUT]
    pw = np.asarray(inputs["attn_p_w"], np.float32)  # [1, E]
    pbv = np.asarray(inputs["attn_p_b"], np.float32) # [1]

    # block-diagonal lhsT [128, 128]: two stacked batches share the PE array
    w1s = np.zeros((128, 128), np.float32)
    w1s[0:64, 0:64] = w1.T
    w1s[64:128, 64:128] = w1.T
    w1s = w1s.astype(bf)
    w2s = np.zeros((128, 128), np.float32)
    w2s[0:64, 0:64] = w2.T
    w2s[64:128, 64:128] = w2.T
    w2s = w2s.astype(bf)
    b1s = np.tile(b1, 2).reshape(128, 1).astype(np.float32)
    b2s = np.tile(b2, 2).reshape(128, 1).astype(np.float32)
    pws = np.tile(pw[0], 2).reshape(128, 1).astype(np.float32)
    mask = np.zeros((128, 2), np.float32)
    mask[:64, 0] = 1.0
    mask[64:, 1] = 1.0
    pb2 = np.full((2, 1), float(pbv.reshape(-1)[0]), np.float32)

    idx_even = np.arange(96) % 64
    idx_odd = (np.arange(96) + 1) % 64

    common = {
        "w1s": w1s, "w2s": w2s, "b1s": b1s, "b2s": b2s,
        "pws": pws, "mask": mask, "pb": pb2,
    }
    in_maps = []
    for c in range(NCORES):
        xs = x[c * BLOC:(c + 1) * BLOC]              # [64, F, E]
        xt = xs.transpose(2, 0, 1)                   # [E, b, F]
        cat = np.concatenate([xt[:, :, idx_even], xt[:, :, idx_odd]], axis=2)
        # [E, b, 192] with b = 2i + t  ->  row p = t*64 + e
        xr = (cat.reshape(E, NPAIR, 2, XW)
                 .transpose(2, 0, 1, 3)
                 .reshape(128, NPAIR * XW)
                 .astype(bf))
        in_maps.append({"xrot": np.ascontiguousarray(xr), **common})
    return in_maps


def run(inputs, trace=False):
    nc = _get_nc()
    in_maps = _prep_in_maps(inputs)
    res = run_bass_kernel_spmd(nc, in_maps, core_ids=list(range(NCORES)),
                               trace=trace)
    out = np.concatenate([res.results[c]["out"] for c in range(NCORES)], axis=0)
    return out.astype(np.float32), res


def kernel(**inputs):
    out, _ = run(inputs, trace=False)
    return out


# revision 8
# speedup vs baseline: 1.1910x; 1.1910x over previous
"""Trainium2 Bass kernel for nn_AFMExpert (AFM attention-over-pairs net).

Math (per batch b):
    had[p, e]  = x[b, i_p, e] * x[b, j_p, e]          P = F*(F-1)/2 pairs
    a          = relu(had @ W1^T + b1)                 [P, NHID]
    logits     = a @ W2^T + b2                         [P, OUT]
    attn       = softmax(logits, axis=pairs)
    afm[e]     = sum_p attn[p, e] * had[p, e]          (OUT == E)
    out[b]     = afm @ pw^T + pb                       [1]

Distribution: pure data parallel, batch sharded 8 ways (64 batches/core),
weights replicated.  No collectives.

Per-core layout ("L1"): two batches stacked on the 128 SBUF partitions
(partition = t*64 + e, t in {0,1}), pair dim + batch-pair dim on the free
axis.  Pairs are enumerated by rotation diagonals d in 1..31 (64 pairs
each) plus d=32 (32 pairs); the host ships x pre-rotated (xrot) so `had`
generation is 3 big DVE tensor_mul ops per 4-batch-pair group, using
stride-tricked access patterns (stride-0 broadcast of the base block, a
stride-2 overlapping-window walk over the rotated block) that run in the
DVE's 2x bf16 perf mode.

Per batch-pair slot:
  mm1 (PE, bf16, block-diag [128,128] lhsT) -> ps1 f32 (2x 2-bank tiles)
  relu evac (1x PSUM read): ACT activation(Relu, bias) or DVE
     tensor_scalar(add bias, max 0) -- the ACT/DVE split is a balance knob
  mm2 -> ps2 f32 (2x 2-bank tiles)
  exp (ACT, 1x): activation(Exp, bias) PSUM -> SBUF *bf16*, softmax
     normalizer Z accumulated free via accum_out
  prod = exp * had: plain tensor_tensor, all-bf16 SBUF => DVE 2x mode
  S    = sum(prod): tensor_scalar(mult 1.0, accum_out), single-src
     all-SBUF bf16 => DVE 4x mode
Everything downstream needs only S and Z: out = sum_e pw*S/Z + pb.

Measured on 8 axon trn2 cores (baseline v1: ~187us).
"""

import os
import sys

for _p in ("/opt/trn_rl_repo", "/opt/pypackages"):
    if os.path.isdir(_p) and _p not in sys.path:
        sys.path.append(_p)

from contextlib import ExitStack

import ml_dtypes
import numpy as np

import concourse.bass as bass
import concourse.mybir as mybir
import concourse.tile as tile
from concourse import bacc
from concourse.bass_utils import run_bass_kernel_spmd
from concourse.dve_ops import AFFINE_MUL_REDUCE

BF16 = mybir.dt.bfloat16
F32 = mybir.dt.float32

B, F, E, NHID, OUT = 512, 64, 64, 64, 64
NCORES = 8
BLOC = B // NCORES          # 64 batches per core
NPAIR = BLOC // 2           # 32 batch-pairs per core
PF = 2048                   # padded pair-block width
NREAL = 2016                # real pairs = F*(F-1)/2
XW = 192                    # xrot row width: 96 even-base + 96 odd-base cols
GN = 4                      # batch-pairs per had group
NG = NPAIR // GN            # 8 groups
PH = 1024                   # half width (2 PSUM banks of f32)

# pairs whose relu evacuation runs on DVE instead of ACT (engine balance)
RELU_DVE_PAIRS = frozenset((3, 7, 11, 15, 19, 23, 27, 31))


def _win_ap(xg, gi_n, start, step, cnt):
    """Overlapping-window AP over the rotated-x tile: [128, gn, cnt, 64]
    where dim 2 walks `cnt` windows of 64 cols spaced `step` apart."""
    w = xg[:, :, start:start + 64].unsqueeze(2).broadcast_to(
        [128, gi_n, cnt, 64])
    lst = w.ap
    lst[2] = [step, cnt]
    w.ap = lst
    return w


def _build_nc():
    # Bacc (not raw Bass): its finalize() runs generate_event_semaphores,
    # which splits multi-wait sync_info onto InstEventSemaphore — the TRN2
    # ISA allows at most 1 sync wait per regular instruction.
    nc = bacc.Bacc(None)

    xrot = nc.declare_dram_parameter("xrot", [128, NPAIR * XW], BF16, isOutput=False)
    w1s_d = nc.declare_dram_parameter("w1s", [128, 128], BF16, isOutput=False)
    w2s_d = nc.declare_dram_parameter("w2s", [128, 128], BF16, isOutput=False)
    b1s_d = nc.declare_dram_parameter("b1s", [128, 1], F32, isOutput=False)
    b2s_d = nc.declare_dram_parameter("b2s", [128, 1], F32, isOutput=False)
    pws_d = nc.declare_dram_parameter("pws", [128, 1], F32, isOutput=False)
    mask_d = nc.declare_dram_parameter("mask", [128, 2], F32, isOutput=False)
    pb_d = nc.declare_dram_parameter("pb", [2, 1], F32, isOutput=False)
    out_d = nc.declare_dram_parameter("out", [BLOC, 1], F32, isOutput=True)

    with tile.TileContext(nc) as tc, ExitStack() as ctx:
        const = ctx.enter_context(tc.tile_pool(name="const", bufs=1))
        hadp = ctx.enter_context(tc.tile_pool(name="hadp", bufs=2))
        aring = ctx.enter_context(tc.tile_pool(name="aring", bufs=2))
        ering = ctx.enter_context(tc.tile_pool(name="ering", bufs=3))
        pring = ctx.enter_context(tc.tile_pool(name="pring", bufs=2))
        small = ctx.enter_context(tc.tile_pool(name="small", bufs=1))
        ps1p = ctx.enter_context(tc.tile_pool(name="ps1p", bufs=2, space="PSUM"))
        ps2p = ctx.enter_context(tc.tile_pool(name="ps2p", bufs=2, space="PSUM"))

        # ---- constants / inputs to SBUF ----
        # xrot is DMA'd per had-group so group 0's had-generation can
        # start early instead of waiting for the full transfer
        xr_tiles = []
        for g in range(NG):
            xr_g = const.tile([128, GN, XW], BF16, tag=f"xr{g}", name=f"xr{g}")
            nc.sync.dma_start(
                out=xr_g[:].rearrange("p a b -> p (a b)"),
                in_=xrot[:, g * GN * XW:(g + 1) * GN * XW])
            xr_tiles.append(xr_g)
        w1s = const.tile([128, 128], BF16, tag="w1s")
        nc.sync.dma_start(out=w1s[:], in_=w1s_d[:, :])
        w2s = const.tile([128, 128], BF16, tag="w2s")
        nc.sync.dma_start(out=w2s[:], in_=w2s_d[:, :])
        b1s = const.tile([128, 1], F32, tag="b1s")
        nc.sync.dma_start(out=b1s[:], in_=b1s_d[:, :])
        b2s = const.tile([128, 1], F32, tag="b2s")
        nc.sync.dma_start(out=b2s[:], in_=b2s_d[:, :])
        pws = const.tile([128, 1], F32, tag="pws")
        nc.sync.dma_start(out=pws[:], in_=pws_d[:, :])
        mask = const.tile([128, 2], F32, tag="mask")
        nc.sync.dma_start(out=mask[:], in_=mask_d[:, :])
        pb = const.tile([2, 1], F32, tag="pb")
        nc.sync.dma_start(out=pb[:], in_=pb_d[:, :])

        # accumulators: Z per (pair, psum-half), S per pair
        Zs2 = small.tile([128, 2 * NPAIR], F32, tag="Zs2")
        Ss = small.tile([128, NPAIR], F32, tag="Ss")

        # ---- had generation: 3 fused DVE ops per group ----
        def had_ops(had_t, g):
            """Thunks for one group's had generation (interleaved between
            the previous group's pairs so the DVE pipelines them)."""
            xg = xr_tiles[g]

            def evens():
                out = had_t[:].rearrange(
                    "p g (d x) -> p g d x", x=128)[:, :, 0:15, 64:128]
                in0 = xg[:, :, 0:64].unsqueeze(2).broadcast_to(
                    [128, GN, 15, 64])
                nc.vector.tensor_mul(out, in0, _win_ap(xg, GN, 2, 2, 15))

            def odds():
                out = had_t[:].rearrange(
                    "p g (d x) -> p g d x", x=128)[:, :, 0:16, 0:64]
                in0 = xg[:, :, 0:64].unsqueeze(2).broadcast_to(
                    [128, GN, 16, 64])
                nc.vector.tensor_mul(out, in0, _win_ap(xg, GN, 96, 2, 16))

            def d32():
                nc.vector.tensor_mul(
                    had_t[:, :, 1984:2016], xg[:, :, 0:32], xg[:, :, 32:64])

            return [evens, odds, d32]

        had_tiles = [hadp.tile([128, GN, PF], BF16, tag="had", name=f"had{g}")
                     for g in range(2)]

        def get_had(g):
            return had_tiles[g % 2]

        def do_pair(i, had_g, li):
            # mm1: block-diag lhsT, 4 chunks into two 2-bank PSUM tiles
            ps1a = ps1p.tile([128, PH], F32, tag="ps1h")
            ps1b = ps1p.tile([128, PH], F32, tag="ps1h")
            for c, pst in ((0, ps1a), (1, ps1a), (2, ps1b), (3, ps1b)):
                w = 480 if c == 3 else 512
                s = 512 * c
                nc.tensor.matmul(pst[:, s - 1024 * (c // 2):s - 1024 * (c // 2) + w],
                                 w1s[:], had_g[:, li, s:s + w],
                                 start=True, stop=True)

            a_sb = aring.tile([128, PF], BF16, tag="a")
            for h, pst in ((0, ps1a), (1, ps1b)):
                w = PH if h == 0 else NREAL - PH
                if i in RELU_DVE_PAIRS:
                    nc.vector.tensor_scalar(
                        out=a_sb[:, PH * h:PH * h + w], in0=pst[:, 0:w],
                        scalar1=b1s[:], scalar2=0.0,
                        op0=mybir.AluOpType.add, op1=mybir.AluOpType.max,
                    )
                else:
                    nc.scalar.activation(
                        a_sb[:, PH * h:PH * h + w], pst[:, 0:w],
                        mybir.ActivationFunctionType.Relu,
                        bias=b1s[:], scale=1.0,
                    )

            # mm2 + exp: exp reads PSUM f32, writes SBUF bf16; softmax
            # normalizer Z accumulated via accum_out
            exp_sb = ering.tile([128, PF], BF16, tag="e")
            for h in range(2):
                ps2 = ps2p.tile([128, PH], F32, tag="ps2h")
                for c in range(2):
                    s = PH * h + 512 * c
                    w = 480 if (h, c) == (1, 1) else 512
                    nc.tensor.matmul(ps2[:, 512 * c:512 * c + w], w2s[:],
                                     a_sb[:, s:s + w], start=True, stop=True)
                w = PH if h == 0 else NREAL - PH
                nc.scalar.activation(
                    exp_sb[:, PH * h:PH * h + w], ps2[:, 0:w],
                    mybir.ActivationFunctionType.Exp,
                    bias=b2s[:], scale=1.0,
                    accum_out=Zs2[:, 2 * i + h:2 * i + h + 1],
                )

            # attn-weighted sum: one fused custom-DVE op
            # out = (exp*1 + 0) * had, accum_out = sum(out) = S
            prod = pring.tile([128, PF], BF16, tag="prod")
            nc.vector._custom_dve(
                AFFINE_MUL_REDUCE,
                out=prod[:, 0:NREAL], in0=exp_sb[:, 0:NREAL],
                in1=had_g[:, li, 0:NREAL],
                s0=1.0, s1=0.0,
                accum_out=Ss[:, i:i + 1],
            )

        # group 0's had-generation runs up front; group g+1's 3 ops are
        # emitted after the prod/S of the first 3 pairs of group g
        for op in had_ops(had_tiles[0], 0):
            op()
        for g in range(NG):
            had_g = get_had(g)
            nxt = had_ops(get_had(g + 1), g + 1) if g + 1 < NG else []
            for li in range(GN):
                do_pair(g * GN + li, had_g, li)
                if li < len(nxt):
                    nxt[li]()

        # ---- finalize ----
        Zs2v = Zs2[:].rearrange("p (i h) -> p i h", h=2)
        Zs = small.tile([128, NPAIR], F32, tag="Zs")
        nc.vector.tensor_add(Zs[:], Zs2v[:, :, 0], Zs2v[:, :, 1])
        rz = small.tile([128, NPAIR], F32, tag="rz")
        nc.vector.reciprocal(rz[:], Zs[:])
        v = small.tile([128, NPAIR], F32, tag="v")
        nc.vector.tensor_mul(v[:], Ss[:], rz[:])
        v2 = small.tile([128, NPAIR], F32, tag="v2")
        nc.vector.tensor_scalar_mul(v2[:], v[:], pws[:])

        # partition-halves reduction via mask matmul: [2, NPAIR]
        fin_ps = ps1p.tile([2, NPAIR], F32, tag="ps1h")
        nc.tensor.matmul(fin_ps[:], mask[:], v2[:], start=True, stop=True)
        fin_sb = small.tile([2, NPAIR], F32, tag="fin")
        nc.vector.tensor_scalar(
            out=fin_sb[:], in0=fin_ps[:], scalar1=pb[:], scalar2=None,
            op0=mybir.AluOpType.add,
        )
        nc.sync.dma_start(
            out=out_d[:].rearrange("(i t) o -> t (i o)", t=2),
            in_=fin_sb[:],
        )

    nc.finalize()
    return nc


_NC = None


def _get_nc():
    global _NC
    if _NC is None:
        _NC = _build_nc()
    return _NC


def _prep_in_maps(inputs):
    bf = ml_dtypes.bfloat16
    x = np.asarray(inputs["x"], np.float32)          # [B, F, E]
    w1 = np.asarray(inputs["attn_w_w"], np.float32)  # [NHID, E]
    b1 = np.asarray(inputs["attn_w_b"], np.float32)  # [NHID]
    w2 = np.asarray(inputs["attn_h_w"], np.float32)  # [OUT, NHID]
    b2 = np.asarray(inputs["attn_h_b"], np.float32)  # [OUT]
    pw = np.asarray(inputs["attn_p_w"], np.float32)  # [1, E]
    pbv = np.asarray(inputs["attn_p_b"], np.float32) # [1]

    # block-diagonal lhsT [128, 128]: two stacked batches share the PE array
    w1s = np.zeros((128, 128), np.float32)
    w1s[0:64, 0:64] = w1.T
    w1s[64:128, 64:128] = w1.T
    w1s = w1s.astype(bf)
    w2s = np.zeros((128, 128), np.float32)
    w2s[0:64, 0:64] = w2.T
    w2s[64:128, 64:128] = w2.T
    w2s = w2s.astype(bf)
    b1s = np.tile(b1, 2).reshape(128, 1).astype(np.float32)
    b2s = np.tile(b2, 2).reshape(128, 1).astype(np.float32)
    pws = np.tile(pw[0], 2).reshape(128, 1).astype(np.float32)
    mask = np.zeros((128, 2), np.float32)
    mask[:64, 0] = 1.0
    mask[64:, 1] = 1.0
    pb2 = np.full((2, 1), float(pbv.reshape(-1)[0]), np.float32)

    idx_even = np.arange(96) % 64
    idx_odd = (np.arange(96) + 1) % 64

    common = {
        "w1s": w1s, "w2s": w2s, "b1s": b1s, "b2s": b2s,
        "pws": pws, "mask": mask, "pb": pb2,
    }
    in_maps = []
    for c in range(NCORES):
        xs = x[c * BLOC:(c + 1) * BLOC]              # [64, F, E]
        xt = xs.transpose(2, 0, 1)                   # [E, b, F]
        cat = np.concatenate([xt[:, :, idx_even], xt[:, :, idx_odd]], axis=2)
        # [E, b, 192] with b = 2i + t  ->  row p = t*64 + e
        xr = (cat.reshape(E, NPAIR, 2, XW)
                 .transpose(2, 0, 1, 3)
                 .reshape(128, NPAIR * XW)
                 .astype(bf))
        in_maps.append({"xrot": np.ascontiguousarray(xr), **common})
    return in_maps


def run(inputs, trace=False):
    nc = _get_nc()
    in_maps = _prep_in_maps(inputs)
    res = run_bass_kernel_spmd(nc, in_maps, core_ids=list(range(NCORES)),
                               trace=trace)
    out = np.concatenate([res.results[c]["out"] for c in range(NCORES)], axis=0)
    return out.astype(np.float32), res


def kernel(**inputs):
    out, _ = run(inputs, trace=False)
    return out


# revision 11
# speedup vs baseline: 1.2201x; 1.0245x over previous
"""Trainium2 Bass kernel for nn_AFMExpert (AFM attention-over-pairs net).

Math (per batch b):
    had[p, e]  = x[b, i_p, e] * x[b, j_p, e]          P = F*(F-1)/2 pairs
    a          = relu(had @ W1^T + b1)                 [P, NHID]
    logits     = a @ W2^T + b2                         [P, OUT]
    attn       = softmax(logits, axis=pairs)
    afm[e]     = sum_p attn[p, e] * had[p, e]          (OUT == E)
    out[b]     = afm @ pw^T + pb                       [1]

Distribution: pure data parallel, batch sharded 8 ways (64 batches/core),
weights replicated.  No collectives.

Per-core layout ("L1"): two batches stacked on the 128 SBUF partitions
(partition = t*64 + e, t in {0,1}), pair dim + batch-pair dim on the free
axis.  Pairs are enumerated by rotation diagonals d in 1..31 (64 pairs
each) plus d=32 (32 pairs); the host ships x pre-rotated (xrot) so `had`
generation is 3 big DVE tensor_mul ops per 4-batch-pair group, using
stride-tricked access patterns (stride-0 broadcast of the base block, a
stride-2 overlapping-window walk over the rotated block) that run in the
DVE's 2x bf16 perf mode.

Per batch-pair slot:
  mm1 (PE, bf16, block-diag [128,128] lhsT) -> ps1 f32 (2x 2-bank tiles)
  relu evac (1x PSUM read): ACT activation(Relu, bias) or DVE
     tensor_scalar(add bias, max 0) -- the ACT/DVE split is a balance knob
  mm2 -> ps2 f32 (2x 2-bank tiles)
  exp (ACT, 1x): activation(Exp, bias) PSUM -> SBUF *bf16*, softmax
     normalizer Z accumulated free via accum_out
  prod = exp * had: plain tensor_tensor, all-bf16 SBUF => DVE 2x mode
  S    = sum(prod): tensor_scalar(mult 1.0, accum_out), single-src
     all-SBUF bf16 => DVE 4x mode
Everything downstream needs only S and Z: out = sum_e pw*S/Z + pb.

Measured on 8 axon trn2 cores (baseline v1: ~187us).
"""

import os
import sys

for _p in ("/opt/trn_rl_repo", "/opt/pypackages"):
    if os.path.isdir(_p) and _p not in sys.path:
        sys.path.append(_p)

from contextlib import ExitStack

import ml_dtypes
import numpy as np

import concourse.bass as bass
import concourse.mybir as mybir
import concourse.tile as tile
from concourse import bacc
from concourse.bass_utils import run_bass_kernel_spmd
from concourse.dve_ops import AFFINE_MUL_REDUCE

BF16 = mybir.dt.bfloat16
F32 = mybir.dt.float32

B, F, E, NHID, OUT = 512, 64, 64, 64, 64
NCORES = 8
BLOC = B // NCORES          # 64 batches per core
NPAIR = BLOC // 2           # 32 batch-pairs per core
PF = 2048                   # padded pair-block width
NREAL = 2016                # real pairs = F*(F-1)/2
XW = 192                    # xrot row width: 96 even-base + 96 odd-base cols
GN = 4                      # batch-pairs per had group
NG = NPAIR // GN            # 8 groups
PH = 1024                   # half width (2 PSUM banks of f32)

# pairs whose relu evacuation runs on DVE instead of ACT (engine balance)
RELU_DVE_PAIRS = frozenset((3, 7, 11, 15, 19, 23, 27, 31))


def _win_ap(xg, gi_n, start, step, cnt):
    """Overlapping-window AP over the rotated-x tile: [128, gn, cnt, 64]
    where dim 2 walks `cnt` windows of 64 cols spaced `step` apart."""
    w = xg[:, :, start:start + 64].unsqueeze(2).broadcast_to(
        [128, gi_n, cnt, 64])
    lst = w.ap
    lst[2] = [step, cnt]
    w.ap = lst
    return w


def _build_nc():
    # Bacc (not raw Bass): its finalize() runs generate_event_semaphores,
    # which splits multi-wait sync_info onto InstEventSemaphore — the TRN2
    # ISA allows at most 1 sync wait per regular instruction.
    nc = bacc.Bacc(None)

    xrot = nc.declare_dram_parameter("xrot", [128, NPAIR * XW], BF16, isOutput=False)
    w1s_d = nc.declare_dram_parameter("w1s", [128, 128], BF16, isOutput=False)
    w2s_d = nc.declare_dram_parameter("w2s", [128, 128], BF16, isOutput=False)
    b1s_d = nc.declare_dram_parameter("b1s", [128, 1], F32, isOutput=False)
    b2s_d = nc.declare_dram_parameter("b2s", [128, 1], F32, isOutput=False)
    pws_d = nc.declare_dram_parameter("pws", [128, 1], F32, isOutput=False)
    mask_d = nc.declare_dram_parameter("mask", [128, 2], F32, isOutput=False)
    pb_d = nc.declare_dram_parameter("pb", [2, 1], F32, isOutput=False)
    out_d = nc.declare_dram_parameter("out", [BLOC, 1], F32, isOutput=True)

    with tile.TileContext(nc) as tc, ExitStack() as ctx:
        const = ctx.enter_context(tc.tile_pool(name="const", bufs=1))
        hadp = ctx.enter_context(tc.tile_pool(name="hadp", bufs=3))
        aring = ctx.enter_context(tc.tile_pool(name="aring", bufs=2))
        ering = ctx.enter_context(tc.tile_pool(name="ering", bufs=3))
        pring = ctx.enter_context(tc.tile_pool(name="pring", bufs=2))
        small = ctx.enter_context(tc.tile_pool(name="small", bufs=1))
        ps1p = ctx.enter_context(tc.tile_pool(name="ps1p", bufs=2, space="PSUM"))
        ps2p = ctx.enter_context(tc.tile_pool(name="ps2p", bufs=2, space="PSUM"))

        # ---- constants / inputs to SBUF ----
        # xrot is DMA'd per had-group so group 0's had-generation can
        # start early instead of waiting for the full transfer
        xr_tiles = []
        for g in range(NG):
            xr_g = const.tile([128, GN, XW], BF16, tag=f"xr{g}", name=f"xr{g}")
            nc.sync.dma_start(
                out=xr_g[:].rearrange("p a b -> p (a b)"),
                in_=xrot[:, g * GN * XW:(g + 1) * GN * XW])
            xr_tiles.append(xr_g)
        w1s = const.tile([128, 128], BF16, tag="w1s")
        nc.sync.dma_start(out=w1s[:], in_=w1s_d[:, :])
        w2s = const.tile([128, 128], BF16, tag="w2s")
        nc.sync.dma_start(out=w2s[:], in_=w2s_d[:, :])
        b1s = const.tile([128, 1], F32, tag="b1s")
        nc.sync.dma_start(out=b1s[:], in_=b1s_d[:, :])
        b2s = const.tile([128, 1], F32, tag="b2s")
        nc.sync.dma_start(out=b2s[:], in_=b2s_d[:, :])
        pws = const.tile([128, 1], F32, tag="pws")
        nc.sync.dma_start(out=pws[:], in_=pws_d[:, :])
        mask = const.tile([128, 2], F32, tag="mask")
        nc.sync.dma_start(out=mask[:], in_=mask_d[:, :])
        pb = const.tile([2, 1], F32, tag="pb")
        nc.sync.dma_start(out=pb[:], in_=pb_d[:, :])

        # accumulators: Z per (pair, psum-half), S per pair
        Zs2 = small.tile([128, 2 * NPAIR], F32, tag="Zs2")
        Ss = small.tile([128, NPAIR], F32, tag="Ss")

        # ---- had generation: 3 fused DVE ops per group ----
        def had_ops(had_t, g):
            """Thunks for one group's had generation (interleaved between
            the previous group's pairs so the DVE pipelines them)."""
            xg = xr_tiles[g]

            def evens():
                out = had_t[:].rearrange(
                    "p g (d x) -> p g d x", x=128)[:, :, 0:15, 64:128]
                in0 = xg[:, :, 0:64].unsqueeze(2).broadcast_to(
                    [128, GN, 15, 64])
                nc.vector.tensor_mul(out, in0, _win_ap(xg, GN, 2, 2, 15))

            def odds():
                out = had_t[:].rearrange(
                    "p g (d x) -> p g d x", x=128)[:, :, 0:16, 0:64]
                in0 = xg[:, :, 0:64].unsqueeze(2).broadcast_to(
                    [128, GN, 16, 64])
                nc.vector.tensor_mul(out, in0, _win_ap(xg, GN, 96, 2, 16))

            def d32():
                nc.vector.tensor_mul(
                    had_t[:, :, 1984:2016], xg[:, :, 0:32], xg[:, :, 32:64])

            return [evens, odds, d32]

        had_tiles = [hadp.tile([128, GN, PF], BF16, tag="had", name=f"had{g}")
                     for g in range(3)]

        def get_had(g):
            return had_tiles[g % 3]

        # per-pair in-flight state (tiles), keyed by pair index
        St = {}

        def st_mm1(i):
            had_g, li = get_had(i // GN), i % GN
            ps1a = ps1p.tile([128, PH], F32, tag="ps1h")
            ps1b = ps1p.tile([128, PH], F32, tag="ps1h")
            for c, pst in ((0, ps1a), (1, ps1a), (2, ps1b), (3, ps1b)):
                w = 480 if c == 3 else 512
                s = 512 * c
                nc.tensor.matmul(pst[:, s - 1024 * (c // 2):s - 1024 * (c // 2) + w],
                                 w1s[:], had_g[:, li, s:s + w],
                                 start=True, stop=True)
            St[i] = {"ps1": (ps1a, ps1b)}

        def st_relu(i):
            a_sb = aring.tile([128, PF], BF16, tag="a")
            for h, pst in ((0, St[i]["ps1"][0]), (1, St[i]["ps1"][1])):
                w = PH if h == 0 else NREAL - PH
                if i in RELU_DVE_PAIRS:
                    nc.vector.tensor_scalar(
                        out=a_sb[:, PH * h:PH * h + w], in0=pst[:, 0:w],
                        scalar1=b1s[:], scalar2=0.0,
                        op0=mybir.AluOpType.add, op1=mybir.AluOpType.max,
                    )
                else:
                    nc.scalar.activation(
                        a_sb[:, PH * h:PH * h + w], pst[:, 0:w],
                        mybir.ActivationFunctionType.Relu,
                        bias=b1s[:], scale=1.0,
                    )
            St[i]["a"] = a_sb

        def st_mm2(i):
            a_sb = St[i]["a"]
            tiles = []
            for h in range(2):
                ps2 = ps2p.tile([128, PH], F32, tag="ps2h")
                tiles.append(ps2)
                for c in range(2):
                    s = PH * h + 512 * c
                    w = 480 if (h, c) == (1, 1) else 512
                    nc.tensor.matmul(ps2[:, 512 * c:512 * c + w], w2s[:],
                                     a_sb[:, s:s + w], start=True, stop=True)
            St[i]["ps2"] = tiles

        def st_exp(i):
            exp_sb = ering.tile([128, PF], BF16, tag="e")
            for h in range(2):
                w = PH if h == 0 else NREAL - PH
                nc.scalar.activation(
                    exp_sb[:, PH * h:PH * h + w], St[i]["ps2"][h][:, 0:w],
                    mybir.ActivationFunctionType.Exp,
                    bias=b2s[:], scale=1.0,
                    accum_out=Zs2[:, 2 * i + h:2 * i + h + 1],
                )
            St[i]["e"] = exp_sb

        def st_prodS(i):
            had_g, li = get_had(i // GN), i % GN
            prod = pring.tile([128, PF], BF16, tag="prod")
            nc.vector._custom_dve(
                AFFINE_MUL_REDUCE,
                out=prod[:, 0:NREAL], in0=St[i]["e"][:, 0:NREAL],
                in1=had_g[:, li, 0:NREAL],
                s0=1.0, s1=0.0,
                accum_out=Ss[:, i:i + 1],
            )
            del St[i]

        # Lag-one software pipeline: per slot s emit
        #   relu(s-1) | exp(s-2) | mm1(s) | mm2(s-1) | had-chunk | prodS(s-3)
        # so every consumer's dependency is at least one slot old — the ACT
        # queue (bottleneck) never waits mid-slot.  Pool-buffer WAR safety:
        # relu/exp are emitted before the mm that recycles their PSUM tile.
        for op in had_ops(had_tiles[0], 0):
            op()
        for s in range(NPAIR + 3):
            if 1 <= s <= NPAIR:
                st_relu(s - 1)
            if 2 <= s <= NPAIR + 1:
                st_exp(s - 2)
            if s < NPAIR:
                st_mm1(s)
            if 1 <= s <= NPAIR:
                st_mm2(s - 1)
            if 3 <= s:
                st_prodS(s - 3)
            if s < NPAIR:
                g, li = s // GN, s % GN
                if li < 3 and g + 1 < NG:
                    had_ops(get_had(g + 1), g + 1)[li]()

        # ---- finalize ----
        Zs2v = Zs2[:].rearrange("p (i h) -> p i h", h=2)
        Zs = small.tile([128, NPAIR], F32, tag="Zs")
        nc.vector.tensor_add(Zs[:], Zs2v[:, :, 0], Zs2v[:, :, 1])
        rz = small.tile([128, NPAIR], F32, tag="rz")
        nc.vector.reciprocal(rz[:], Zs[:])
        v = small.tile([128, NPAIR], F32, tag="v")
        nc.vector.tensor_mul(v[:], Ss[:], rz[:])
        v2 = small.tile([128, NPAIR], F32, tag="v2")
        nc.vector.tensor_scalar_mul(v2[:], v[:], pws[:])

        # partition-halves reduction via mask matmul: [2, NPAIR]
        fin_ps = ps1p.tile([2, NPAIR], F32, tag="ps1h")
        nc.tensor.matmul(fin_ps[:], mask[:], v2[:], start=True, stop=True)
        fin_sb = small.tile([2, NPAIR], F32, tag="fin")
        nc.vector.tensor_scalar(
            out=fin_sb[:], in0=fin_ps[:], scalar1=pb[:], scalar2=None,
            op0=mybir.AluOpType.add,
        )
        nc.sync.dma_start(
            out=out_d[:].rearrange("(i t) o -> t (i o)", t=2),
            in_=fin_sb[:],
        )

    nc.finalize()
    return nc


_NC = None


def _get_nc():
    global _NC
    if _NC is None:
        _NC = _build_nc()
    return _NC


def _prep_in_maps(inputs):
    bf = ml_dtypes.bfloat16
    x = np.asarray(inputs["x"], np.float32)          # [B, F, E]
    w1 = np.asarray(inputs["attn_w_w"], np.float32)  # [NHID, E]
    b1 = np.asarray(inputs["attn_w_b"], np.float32)  # [NHID]
    w2 = np.asarray(inputs["attn_h_w"], np.float32)  # [OUT, NHID]
    b2 = np.asarray(inputs["attn_h_b"], np.float32)  # [OUT]
    pw = np.asarray(inputs["attn_p_w"], np.float32)  # [1, E]
    pbv = np.asarray(inputs["attn_p_b"], np.float32) # [1]

    # block-diagonal lhsT [128, 128]: two stacked batches share the PE array
    w1s = np.zeros((128, 128), np.float32)
    w1s[0:64, 0:64] = w1.T
    w1s[64:128, 64:128] = w1.T
    w1s = w1s.astype(bf)
    w2s = np.zeros((128, 128), np.float32)
    w2s[0:64, 0:64] = w2.T
    w2s[64:128, 64:128] = w2.T
    w2s = w2s.astype(bf)
    b1s = np.tile(b1, 2).reshape(128, 1).astype(np.float32)
    b2s = np.tile(b2, 2).reshape(128, 1).astype(np.float32)
    pws = np.tile(pw[0], 2).reshape(128, 1).astype(np.float32)
    mask = np.zeros((128, 2), np.float32)
    mask[:64, 0] = 1.0
    mask[64:, 1] = 1.0
    pb2 = np.full((2, 1), float(pbv.reshape(-1)[0]), np.float32)

    idx_even = np.arange(96) % 64
    idx_odd = (np.arange(96) + 1) % 64

    common = {
        "w1s": w1s, "w2s": w2s, "b1s": b1s, "b2s": b2s,
        "pws": pws, "mask": mask, "pb": pb2,
    }
    in_maps = []
    for c in range(NCORES):
        xs = x[c * BLOC:(c + 1) * BLOC]              # [64, F, E]
        xt = xs.transpose(2, 0, 1)                   # [E, b, F]
        cat = np.concatenate([xt[:, :, idx_even], xt[:, :, idx_odd]], axis=2)
        # [E, b, 192] with b = 2i + t  ->  row p = t*64 + e
        xr = (cat.reshape(E, NPAIR, 2, XW)
                 .transpose(2, 0, 1, 3)
                 .reshape(128, NPAIR * XW)
                 .astype(bf))
        in_maps.append({"xrot": np.ascontiguousarray(xr), **common})
    return in_maps


def run(inputs, trace=False):
    nc = _get_nc()
    in_maps = _prep_in_maps(inputs)
    res = run_bass_kernel_spmd(nc, in_maps, core_ids=list(range(NCORES)),
                               trace=trace)
    out = np.concatenate([res.results[c]["out"] for c in range(NCORES)], axis=0)
    return out.astype(np.float32), res


def kernel(**inputs):
    out, _ = run(inputs, trace=False)
    return out
